# revision 22
# baseline (speedup 1.0000x reference)
"""Trainium2 Bass kernel for nn_CrossAttentionLayer (ragged cross-attention + MLP).

Sharding: 64 ragged segments -> 8 cores x 8 slots. Slot i has shared caps
(nd_cap, ns_cap) = max counts over the 8 cores (same compiled program on every
core). All matmul datapath is bf16 (PSUM accumulates fp32); only the softmax
denominator/reciprocal and the output stay fp32.

Per slot: Q/K projected channel-major ([chan, tok]); V projected directly into
natural [tok, chan] layout (src tile slices are the stationary operand, Wv the
moving one) with the bias added by a rank-1 ones matmul. Scores run in scoresT
orientation [src, dst] with the padding mask as a per-partition exp bias; the
four heads' score matmuls sit in disjoint 32-row PE quadrants. The softmax
denominator is computed by pre-summing exp tiles over j-blocks (gpsimd), then
one banded ones-matmul per head replicates each head's denominator across its
32 partitions; normalization is reciprocal_approx_fast + multiply on DVE.
Merge conv + BN are folded into the MLP weights on the host; the residual add
and final bias are fused in one DVE scalar_tensor_tensor.

Engine balance: ACT = exp only; DVE = PSUM-reading elementwise; GPSIMD =
SBUF-only presums.
"""
import math
import sys
from contextlib import ExitStack

import numpy as np

try:
    import concourse.bass as bass
except ImportError:
    sys.path.insert(0, "/opt/trn_rl_repo")
    import concourse.bass as bass

import concourse.tile as tile
from concourse import bacc, mybir
from concourse.bass_utils import run_bass_kernel_spmd

F32 = mybir.dt.float32
BF16 = mybir.dt.bfloat16

B = 64
LMAX = 512
H = 256          # h_dim
C = 128          # h_div
HEADS = 4
DH = 32
NCORES = 8
SEGS = 8         # slots per core
NPB = 7          # per-partition bias columns: bq,bk,bv(unused),b1a,b1b,b2a,b2b
MASK_NEG = -20000.0

# Slot caps (same across cores; set by host_prep via plan_slots).
ND_CAPS = [512] * SEGS       # dst cols per slot (multiple of 16)
NS_CAPS = [512] * SEGS       # src cols per slot (multiple of 128)


def _roundup(x, m):
    return ((int(x) + m - 1) // m) * m


def plan_slots(ns, nd):
    """Assign 64 segments to an 8x8 (core, slot) grid minimizing padded work.

    Returns (assign[core][slot] -> seg index, nd_caps, ns_caps).
    """
    rng = np.random.default_rng(1234)

    def slot_cost(group):
        ndc = _roundup(max(nd[g] for g in group), 16)
        nsc = _roundup(max(ns[g] for g in group), 128)
        return 16 * ndc + 5 * nsc + 5 * ndc * nsc / 128.0

    order = np.argsort(-(ns * nd))
    groups = [list(order[i * NCORES:(i + 1) * NCORES]) for i in range(SEGS)]
    costs = [slot_cost(g) for g in groups]
    for _ in range(20000):
        a, b = rng.integers(0, SEGS, 2)
        if a == b:
            continue
        ia, ib = rng.integers(0, NCORES, 2)
        ga, gb = groups[a][:], groups[b][:]
        ga[ia], gb[ib] = groups[b][ib], groups[a][ia]
        ca, cb = slot_cost(ga), slot_cost(gb)
        if ca + cb < costs[a] + costs[b] - 1e-9:
            groups[a], groups[b] = ga, gb
            costs[a], costs[b] = ca, cb
    # big slots first: their pipelines overlap best when work is plentiful
    slot_order = np.argsort(-np.array(costs))
    groups = [groups[i] for i in slot_order]
    assign = [[groups[i][c] for i in range(SEGS)] for c in range(NCORES)]
    nd_caps = [_roundup(max(nd[g] for g in groups[i]), 16) for i in range(SEGS)]
    ns_caps = [_roundup(max(ns[g] for g in groups[i]), 128) for i in range(SEGS)]
    return assign, nd_caps, ns_caps


def host_prep(inputs):
    global ND_CAPS, NS_CAPS
    src_h = np.asarray(inputs['src_h'], np.float32)
    dst_h = np.asarray(inputs['dst_h'], np.float32)
    ns = np.asarray(inputs['src_num_verts']).astype(np.int64)
    nd = np.asarray(inputs['dst_num_verts']).astype(np.int64)
    soff = np.concatenate([[0], np.cumsum(ns)[:-1]])
    doff = np.concatenate([[0], np.cumsum(nd)[:-1]])

    assign, nd_caps, ns_caps = plan_slots(ns, nd)
    ND_CAPS = nd_caps
    NS_CAPS = ns_caps
    td = int(np.sum(nd_caps))
    ts = int(np.sum(ns_caps))
    dcol = np.concatenate([[0], np.cumsum(nd_caps)[:-1]]).astype(int)
    scol = np.concatenate([[0], np.cumsum(ns_caps)[:-1]]).astype(int)
    njs = [c // 128 for c in ns_caps]
    moff = np.concatenate([[0], np.cumsum(njs)[:-1]]).astype(int)
    nmask = int(np.sum(njs))

    perm = np.empty(C, np.int64)
    for chat in range(C):
        h, d = divmod(chat, DH)
        perm[chat] = d * HEADS + h
    s = 1.0 / math.sqrt(DH)

    f32 = lambda k: np.asarray(inputs[k], np.float32)
    Wq, bq = f32('Wq'), f32('bq')
    Wk, bk = f32('Wk'), f32('bk')
    Wv, bv = f32('Wv'), f32('bv')
    Wm, bm = f32('Wm'), f32('bm')
    W1, b1 = f32('W1'), f32('b1')
    W2, b2 = f32('W2'), f32('b2')
    g1, be1, rm1, rv1 = f32('g1'), f32('be1'), f32('rm1'), f32('rv1')
    g2, be2, rm2, rv2 = f32('g2'), f32('be2'), f32('rm2'), f32('rv2')

    WqT = np.ascontiguousarray((Wq[perm] * s).T)          # [256,128]
    bq_s = bq[perm] * s
    WkT = np.ascontiguousarray(Wk[perm].T)
    bk_r = bk[perm]
    WvT = np.ascontiguousarray(Wv[perm].T)                # [256,128] (rhs for v-nat)
    bv_r = bv[perm]
    Wm_p = Wm[:, perm]
    a1 = g1 / np.sqrt(rv1 + 1e-5)
    W1_f = W1 * a1[:, None]
    b1_f = b1 * a1 + be1 - rm1 * a1
    a2 = g2 / np.sqrt(rv2 + 1e-5)
    W2_f = W2 * a2[:, None]
    b2_f = b2 * a2 + be2 - rm2 * a2
    W1m_p = W1_f[:, H:] @ Wm_p
    b1_p = b1_f + W1_f[:, H:] @ bm
    W1T = np.ascontiguousarray(np.concatenate([W1_f[:, :H], W1m_p], axis=1).T)  # [384,256]
    W2T = np.ascontiguousarray(W2_f.T)                    # [256,256]

    pbias = np.zeros((128, NPB), np.float32)
    pbias[:, 0] = bq_s
    pbias[:, 1] = bk_r
    pbias[:, 2] = bv_r            # unused on device (bias via ones matmul)
    pbias[:, 3] = b1_p[:128]
    pbias[:, 4] = b1_p[128:]
    pbias[:, 5] = b2_f[:128]
    pbias[:, 6] = b2_f[128:]

    bvtile = np.tile(bv_r, 4)[None, :].repeat(128, axis=0).astype(np.float32)  # [128, 512]
    onespad = np.zeros((128, HEADS, 128), np.float32)
    for h in range(HEADS):
        onespad[:, h, h * DH:(h + 1) * DH] = 1.0

    cores = []
    for c in range(NCORES):
        dstT = np.zeros((H, td), np.float32)
        srcT = np.zeros((H, ts), np.float32)
        maskb = np.full((128, nmask), MASK_NEG, np.float32)
        for i in range(SEGS):
            g = assign[c][i]
            dstT[:, dcol[i]:dcol[i] + nd[g]] = dst_h[doff[g]:doff[g] + nd[g]].T
            srcT[:, scol[i]:scol[i] + ns[g]] = src_h[soff[g]:soff[g] + ns[g]].T
            for j in range(njs[i]):
                valid = max(0, min(128, int(ns[g]) - j * 128))
                maskb[:valid, moff[i] + j] = 0.0
        cores.append(dict(dstT=dstT, srcT=srcT, maskb=maskb))

    shared = dict(WqT=WqT, WkT=WkT, WvT=WvT, W1T=W1T, W2T=W2T, pbias=pbias,
                  bvtile=bvtile, onespad=onespad)
    meta = dict(nd=nd, doff=doff, assign=assign, dcol=dcol, td=td)
    return cores, shared, meta


def _bf16(a):
    import ml_dtypes
    return np.asarray(np.asarray(a, np.float32), dtype=ml_dtypes.bfloat16)


def declare_tensors(nc):
    td = int(np.sum(ND_CAPS))
    ts = int(np.sum(NS_CAPS))
    nmask = sum(c // 128 for c in NS_CAPS)
    aps = {}
    aps['dstT'] = nc.dram_tensor("dstT", [H, td], BF16, kind="ExternalInput").ap()
    aps['srcT'] = nc.dram_tensor("srcT", [H, ts], BF16, kind="ExternalInput").ap()
    aps['maskb'] = nc.dram_tensor("maskb", [128, nmask], F32, kind="ExternalInput").ap()
    aps['WqT'] = nc.dram_tensor("WqT", [H, C], BF16, kind="ExternalInput").ap()
    aps['WkT'] = nc.dram_tensor("WkT", [H, C], BF16, kind="ExternalInput").ap()
    aps['WvT'] = nc.dram_tensor("WvT", [H, C], BF16, kind="ExternalInput").ap()
    aps['W1T'] = nc.dram_tensor("W1T", [H + C, H], BF16, kind="ExternalInput").ap()
    aps['W2T'] = nc.dram_tensor("W2T", [H, H], BF16, kind="ExternalInput").ap()
    aps['pbias'] = nc.dram_tensor("pbias", [128, NPB], F32, kind="ExternalInput").ap()
    aps['bvtile'] = nc.dram_tensor("bvtile", [128, 512], BF16, kind="ExternalInput").ap()
    aps['onespad'] = nc.dram_tensor("onespad", [128, HEADS * 128], BF16, kind="ExternalInput").ap()
    aps['outT'] = nc.dram_tensor("outT", [H, td], F32, kind="ExternalOutput").ap()
    return aps


def build_body(ctx: ExitStack, tc: tile.TileContext, aps, pfx=""):
    nc = tc.nc
    td = int(np.sum(ND_CAPS))
    ts = int(np.sum(NS_CAPS))
    njs = [c // 128 for c in NS_CAPS]
    dcol = np.concatenate([[0], np.cumsum(ND_CAPS)[:-1]]).astype(int)
    scol = np.concatenate([[0], np.cumsum(NS_CAPS)[:-1]]).astype(int)
    moff = np.concatenate([[0], np.cumsum(njs)[:-1]]).astype(int)

    wp = ctx.enter_context(tc.tile_pool(name=pfx + "wp", bufs=1))
    inp = ctx.enter_context(tc.tile_pool(name=pfx + "inp", bufs=1))
    qkv = ctx.enter_context(tc.tile_pool(name=pfx + "qkv", bufs=2))
    att = ctx.enter_context(tc.tile_pool(name=pfx + "att", bufs=2))
    mls = ctx.enter_context(tc.tile_pool(name=pfx + "mls", bufs=2))
    gp = ctx.enter_context(tc.tile_pool(name=pfx + "gp", bufs=2, space="PSUM"))
    scp = ctx.enter_context(tc.tile_pool(name=pfx + "scp", bufs=2, space="PSUM"))
    mdp = ctx.enter_context(tc.tile_pool(name=pfx + "mdp", bufs=1, space="PSUM"))

    # --- weights (small, load first) ---
    wq = wp.tile([128, 2, C], BF16, tag="wq")
    wk = wp.tile([128, 2, C], BF16, tag="wk")
    wv = wp.tile([128, 2, C], BF16, tag="wv")
    w1 = wp.tile([128, 3, H], BF16, tag="w1")
    w2 = wp.tile([128, 2, H], BF16, tag="w2")
    pb = wp.tile([128, NPB], F32, tag="pb")
    maskb_t = wp.tile([128, sum(njs)], F32, tag="maskb")
    bvtile_t = wp.tile([128, 512], BF16, tag="bvtile")
    onespad = wp.tile([128, HEADS, 128], BF16, tag="onespad")
    dst_t = [inp.tile([128, td], BF16, tag=f"dst{a}", name=f"dst{a}") for a in range(2)]
    src_t = [inp.tile([128, ts], BF16, tag=f"src{a}", name=f"src{a}") for a in range(2)]
    # order: what slot 0's first matmuls need arrives first; the big input
    # halves stream per-slot so slot 0 never waits on later slots' columns
    def chunk_dma(i):
        d0, d1 = int(dcol[i]), int(dcol[i]) + int(ND_CAPS[i])
        s0, s1 = int(scol[i]), int(scol[i]) + int(NS_CAPS[i])
        for a in range(2):
            nc.sync.dma_start(out=dst_t[a][:, d0:d1],
                              in_=aps['dstT'][a * 128:(a + 1) * 128, d0:d1])
            nc.sync.dma_start(out=src_t[a][:, s0:s1],
                              in_=aps['srcT'][a * 128:(a + 1) * 128, s0:s1])

    for a in range(2):
        nc.sync.dma_start(out=wq[:, a, :], in_=aps['WqT'][a * 128:(a + 1) * 128, :])
    chunk_dma(0)
    nc.sync.dma_start(out=pb[:], in_=aps['pbias'][:])
    for a in range(2):
        nc.sync.dma_start(out=wk[:, a, :], in_=aps['WkT'][a * 128:(a + 1) * 128, :])
        nc.sync.dma_start(out=wv[:, a, :], in_=aps['WvT'][a * 128:(a + 1) * 128, :])
    nc.sync.dma_start(out=bvtile_t[:], in_=aps['bvtile'][:])
    nc.sync.dma_start(out=maskb_t[:], in_=aps['maskb'][:])
    nc.sync.dma_start(out=onespad[:], in_=aps['onespad'].rearrange("p (h c) -> p h c", h=HEADS))
    chunk_dma(1)
    for a in range(3):
        nc.sync.dma_start(out=w1[:, a, :], in_=aps['W1T'][a * 128:(a + 1) * 128, :])
    for a in range(2):
        nc.sync.dma_start(out=w2[:, a, :], in_=aps['W2T'][a * 128:(a + 1) * 128, :])
    for i in range(2, SEGS):
        chunk_dma(i)

    qkv_tiles = {}

    def emit_proj(i):
        ndc, nsc, nj = int(ND_CAPS[i]), int(NS_CAPS[i]), njs[i]
        do, so = int(dcol[i]), int(scol[i])
        ps_q = gp.tile([128, 512], F32, tag="gp", name=f"psq{i}")
        for a in range(2):
            nc.tensor.matmul(ps_q[:, :ndc], wq[:, a, :], dst_t[a][:, do:do + ndc],
                             start=(a == 0), stop=(a == 1))
        q_t = qkv.tile([128, 512], BF16, tag="q", name=f"q{i}")
        nc.vector.tensor_scalar_add(q_t[:, :ndc], ps_q[:, :ndc], pb[:, 0:1])

        ps_k = gp.tile([128, 512], F32, tag="gp", name=f"psk{i}")
        for a in range(2):
            nc.tensor.matmul(ps_k[:, :nsc], wk[:, a, :], src_t[a][:, so:so + nsc],
                             start=(a == 0), stop=(a == 1))
        k_t = qkv.tile([128, 512], BF16, tag="k", name=f"k{i}")
        nc.vector.tensor_scalar_add(k_t[:, :nsc], ps_k[:, :nsc], pb[:, 1:2])

        # V directly in natural [tok, chan] layout, one 128-token block per j;
        # bv added during the PSUM->SBUF copy
        ps_v = gp.tile([128, 512], F32, tag="gp", name=f"psv{i}")
        for b in range(nj):
            blk = slice(b * 128, (b + 1) * 128)
            for a in range(2):
                nc.tensor.matmul(ps_v[:, blk],
                                 src_t[a][:, so + b * 128:so + (b + 1) * 128],
                                 wv[:, a, :], start=(a == 0), stop=(a == 1))
        v_t = qkv.tile([128, 512], BF16, tag="v", name=f"v{i}")
        nc.vector.tensor_add(v_t[:, :nj * 128], ps_v[:, :nj * 128],
                             bvtile_t[:, :nj * 128])
        qkv_tiles[i] = (q_t, k_t, v_t)

    def emit_attn(i):
        ndc, nsc, nj = int(ND_CAPS[i]), int(NS_CAPS[i]), njs[i]
        do, so, mo = int(dcol[i]), int(scol[i]), int(moff[i])
        q_t, k_t, v_t = qkv_tiles.pop(i)

        # ---------- attention ----------
        # presum of exp over j-blocks is pipelined into the scores/exp loop:
        # s01 (gpsimd) right after exp(j=1), tail add (DVE) after the last exp.
        e_all = att.tile([128, 4, HEADS, 512], BF16, tag="E", name=f"E{i}", bufs=2)
        es = att.tile([128, HEADS, 512], BF16, tag="S", name=f"S{i}", bufs=2)
        s01 = att.tile([128, HEADS, 512], BF16, tag="s01", name=f"s01_{i}", bufs=2)
        ps_msg = mdp.tile([128, 512], F32, tag="msg", name=f"msg{i}")

        def emit_msg(j):
            # per-band accumulation groups, interleaved j-outer so msg(j)
            # overlaps exp(j+1); groups are partition-disjoint per band
            for h in range(HEADS):
                nc.tensor.matmul(
                    ps_msg[32 * h:32 * h + 32, :ndc],
                    v_t[:, j * 128 + 32 * h:j * 128 + 32 * h + 32],
                    e_all[:, j, h, :ndc],
                    start=(j == 0), stop=(j == nj - 1),
                    tile_position=(0, 32 * h), skip_group_check=True)

        for j in range(nj):
            for hp in range(2):
                ps_sc = scp.tile([128, 2, 512], F32, tag="sc", name=f"sc{i}_{j}_{hp}")
                for hh in range(2):
                    h = 2 * hp + hh
                    nc.tensor.matmul(
                        ps_sc[:, hh, :ndc],
                        k_t[32 * h:32 * h + 32, j * 128:(j + 1) * 128],
                        q_t[32 * h:32 * h + 32, :ndc],
                        start=True, stop=True, tile_position=(32 * h, 0))
                nc.scalar.activation(e_all[:, j, 2 * hp:2 * hp + 2, :ndc],
                                     ps_sc[:, :, :ndc],
                                     mybir.ActivationFunctionType.Exp,
                                     bias=maskb_t[:, mo + j:mo + j + 1])
            if j >= 1:
                emit_msg(j - 1)
            if j == 1 and nj > 2:
                nc.gpsimd.tensor_add(s01[:, :, :ndc], e_all[:, 0, :, :ndc],
                                     e_all[:, 1, :, :ndc])
        emit_msg(nj - 1)

        if nj == 2:
            nc.vector.tensor_add(es[:, :, :ndc], e_all[:, 0, :, :ndc],
                                 e_all[:, 1, :, :ndc])
        elif nj == 3:
            nc.vector.tensor_add(es[:, :, :ndc], s01[:, :, :ndc],
                                 e_all[:, 2, :, :ndc])
        else:
            s23 = att.tile([128, HEADS, 512], BF16, tag="s23", name=f"s23_{i}", bufs=2)
            nc.vector.tensor_add(s23[:, :, :ndc], e_all[:, 2, :, :ndc],
                                 e_all[:, 3, :, :ndc])
            nc.vector.tensor_add(es[:, :, :ndc], s01[:, :, :ndc],
                                 s23[:, :, :ndc])

        ps_den = mdp.tile([128, 512], F32, tag="den", name=f"den{i}")
        for h in range(HEADS):
            nc.tensor.matmul(ps_den[:, :ndc], onespad[:, h, :], es[:, h, :ndc],
                             start=(h == 0), stop=(h == 3))
        r_sb = att.tile([128, 512], F32, tag="rsb", name=f"rsb{i}", bufs=2)
        nc.vector.reciprocal_approx_fast(out=r_sb[:, :ndc], in_=ps_den[:, :ndc])
        msgn = att.tile([128, 512], BF16, tag="msgn", name=f"msgn{i}", bufs=2)
        nc.vector.tensor_mul(msgn[:, :ndc], ps_msg[:, :ndc], r_sb[:, :ndc])
        return msgn

    def emit_tail(i, msgn):
        # MLP (merge conv + BN folded); deferred behind the next slot's head
        # so its PE work never waits on this slot's recip/mult
        ndc = int(ND_CAPS[i])
        do = int(dcol[i])
        y1 = [None, None]
        for o in range(2):
            ps_y = gp.tile([128, 512], F32, tag="gp", name=f"psy{i}_{o}")
            rhs_list = [dst_t[0][:, do:do + ndc], dst_t[1][:, do:do + ndc],
                        msgn[:, :ndc]]
            for kk in range(3):
                nc.tensor.matmul(ps_y[:, :ndc], w1[:, kk, o * 128:(o + 1) * 128],
                                 rhs_list[kk], start=(kk == 0), stop=(kk == 2))
            y1_t = mls.tile([128, 512], BF16, tag=f"y1_{o}", name=f"y1_{i}_{o}")
            nc.vector.tensor_scalar(y1_t[:, :ndc], ps_y[:, :ndc],
                                    pb[:, 3 + o:4 + o], 0.0,
                                    op0=mybir.AluOpType.add,
                                    op1=mybir.AluOpType.max)
            y1[o] = y1_t
        for o in range(2):
            ps_z = gp.tile([128, 512], F32, tag="gp", name=f"psz{i}_{o}")
            for kk in range(2):
                nc.tensor.matmul(ps_z[:, :ndc], w2[:, kk, o * 128:(o + 1) * 128],
                                 y1[kk][:, :ndc], start=(kk == 0), stop=(kk == 1))
            out_sb = mls.tile([128, 512], F32, tag=f"out_{o}", name=f"out_{i}_{o}")
            nc.vector.scalar_tensor_tensor(
                out_sb[:, :ndc], ps_z[:, :ndc], pb[:, 5 + o:6 + o],
                dst_t[o][:, do:do + ndc],
                op0=mybir.AluOpType.add, op1=mybir.AluOpType.add)
            nc.sync.dma_start(out=aps['outT'][o * 128:(o + 1) * 128, do:do + ndc],
                              in_=out_sb[:, :ndc])

    # 3-stage pipeline: attn(i) | proj(i+1) | mlp(i). proj(i+1) sits between
    # attn(i) and mlp(i) so the gp PSUM rotation never waits on the tail's
    # DVE ops, and mlp(i)'s msgn is ready (recip/mult ran during proj).
    emit_proj(0)
    for i in range(SEGS):
        msgn = emit_attn(i)
        if i + 1 < SEGS:
            emit_proj(i + 1)
        emit_tail(i, msgn)


def build_nc(reps=1):
    nc = bacc.Bacc("TRN2", target_bir_lowering=False, debug=False,
                   enable_asserts=True, num_devices=NCORES)
    aps = declare_tensors(nc)
    with tile.TileContext(nc) as tc:
        for rep in range(reps):
            with ExitStack() as ctx:
                build_body(ctx, tc, aps, pfx=f"r{rep}" if rep else "")
    nc.compile()
    return nc


def in_map(core, shared):
    m = dict(dstT=_bf16(core['dstT']), srcT=_bf16(core['srcT']),
             maskb=core['maskb'])
    m['WqT'] = _bf16(shared['WqT'])
    m['WkT'] = _bf16(shared['WkT'])
    m['WvT'] = _bf16(shared['WvT'])
    m['W1T'] = _bf16(shared['W1T'])
    m['W2T'] = _bf16(shared['W2T'])
    m['pbias'] = shared['pbias']
    m['bvtile'] = _bf16(shared['bvtile'])
    m['onespad'] = _bf16(shared['onespad'].reshape(128, HEADS * 128))
    return m


def assemble(outTs, meta):
    nd = meta['nd']
    doff = meta['doff']
    assign = meta['assign']
    dcol = meta['dcol']
    out = np.empty((int(nd.sum()), H), np.float32)
    for c in range(NCORES):
        for i in range(SEGS):
            g = assign[c][i]
            out[doff[g]:doff[g] + nd[g]] = \
                outTs[c][:, dcol[i]:dcol[i] + nd[g]].T
    return out


def kernel(**inputs):
    cores, shared, meta = host_prep(inputs)
    nc = build_nc()
    in_maps = [in_map(cores[c], shared) for c in range(NCORES)]
    res = run_bass_kernel_spmd(nc, in_maps, core_ids=list(range(NCORES)))
    outTs = [np.asarray(res.results[c]["outT"], np.float32) for c in range(NCORES)]
    return assemble(outTs, meta)
